# revision 21
# baseline (speedup 1.0000x reference)
"""NodeClustering (vq_codebook) Trainium2 kernel — v3.

Math (per batch element b, P=16384 points, C=256 channels, K=8 clusters):
  nodes = F_p @ proj_w.T + proj_b
  3 iterations of: sim = l2(nodes) @ l2(centers).T ; assign = argmax;
                   centers = segment_mean(nodes)
  weights = softmax(10 * l2(nodes) @ l2(centers).T)
  out = (weights@centers + F_p) @ refine_w.T + refine_b

Restructuring so `nodes` is never materialized:
  nodes[p] . cn_k       = F_p[p] . g_k + h_k   (g_k = proj_w.T @ cn_k, h_k = proj_b . cn_k)
  segment_mean(nodes)   = segment_mean(F_p) @ proj_w.T + proj_b
  out                   = F_p @ refine_w.T + weights @ [centers @ refine_w.T ; refine_b]
  iteration argmax is scale-invariant -> skip 1/||nodes_p||; final softmax
  needs inv10_p = 10/||nodes_p||, with
  ||nodes_p||^2 = ||F_p[p] @ proj_w.T||^2 + 2*F_p[p].u + ||proj_b||^2,  u = proj_w.T @ proj_b

v3 performance structure (v2 at 614us was bound by per-chunk DVE/ACT/GPSIMD
small ops, which cost ~400-900ns each on HW regardless of size):
  - fp16 residents: fT (C-major, [128, 2, P]) + natb (point-major + baked
    ones column for counts). Iterations re-read nothing from HBM.
  - Chunks processed in groups of GRP=8. Sims for a group land in ONE psum
    tile [128, 8, 9]; h is added by a single ones x hrow matmul per group
    (PE) instead of per-chunk vector adds; argmax+onehot is 4 batched DVE
    ops per group (pairwise-max tree + one is_ge against a stride-0
    broadcast of the max), not 3 ops per chunk.
  - Norm pass (nodes + square-accumulate) split across iterations 2 and 3
    (it only depends on F_p), sharing the sim LDWEIGHTS.
  - Final softmax fully batched per group: scale-by-inv10 (broadcast mult),
    exp, avg-pool denominator, reciprocal, normalize — 5 vector ops per 8
    chunks. Phase 5 runs as sweep A (sims+softmax) then sweep B (weight
    transposes + refine + cluster matmul + output) within 8 PSUM banks.
  - f32->fp16 casts batched per 8-chunk DMA group, split GPSIMD/DVE.

Sharding: pure data parallel, core i <- batch element i (B=8, 8 cores).
"""

import sys
import numpy as np

sys.path.insert(0, "/opt/trn_rl_repo")

import concourse.bass as bass
import concourse.bacc as bacc
import concourse.mybir as mybir
import concourse.tile as tile
from concourse._compat import get_trn_type
from concourse.bass_utils import axon_active
from concourse.masks import make_identity
from concourse.bass_utils import run_bass_kernel_spmd

P = 16384
C = 256
NK = 8
NUM_ITERS = 3
EPS = 1e-12
N_CORES = 8
NCHUNK = P // 128
GRP = 8          # chunks per sim/argmax group (= chunks per input/output DMA)

F32 = mybir.dt.float32
F16 = mybir.dt.float16
AF = mybir.ActivationFunctionType
ALU = mybir.AluOpType


def build_bass(p=P):
    nchunk = p // 128
    ngrp = nchunk // GRP
    idx = list(np.linspace(0, p - 1, NK).astype(np.int64))
    nc = bacc.Bacc(
        get_trn_type() or "TRN2",
        target_bir_lowering=False,
        debug=not axon_active(),
        num_devices=N_CORES,
    )

    fp = nc.dram_tensor("fp", [p, C], F32, kind="ExternalInput")
    pw = nc.dram_tensor("pw", [C, C], F32, kind="ExternalInput")
    pb = nc.dram_tensor("pb", [C], F32, kind="ExternalInput")
    rw = nc.dram_tensor("rw", [C, C], F32, kind="ExternalInput")
    rb = nc.dram_tensor("rb", [C], F32, kind="ExternalInput")
    out = nc.dram_tensor("out", [p, C], F32, kind="ExternalOutput")

    fp_v = fp[:].rearrange("(n p) c -> p n c", p=128)
    out_v = out[:].rearrange("(n p) c -> p n c", p=128)

    with tile.TileContext(nc) as tc:
        with (
            tc.tile_pool(name="res", bufs=1) as res,
            tc.tile_pool(name="natp", bufs=2) as natp,
            tc.tile_pool(name="outp", bufs=2) as outp,
            tc.tile_pool(name="sml", bufs=4) as sml,
            tc.tile_pool(name="scr", bufs=2) as scr,
            tc.tile_pool(name="itp", bufs=2) as itp,
            tc.tile_pool(name="ps_tp", bufs=2, space="PSUM") as ps_tp,    # fp16: transposes / wTg
            tc.tile_pool(name="ps_acc", bufs=3, space="PSUM") as ps_acc,  # f32: opN/opR + small mm
            tc.tile_pool(name="ps_sg", bufs=2, space="PSUM") as ps_sg,    # sim groups
            tc.tile_pool(name="ps_s", bufs=1, space="PSUM") as ps_s,      # S accumulator
        ):
            # ---------------- phase 0: constants + weights ----------------
            identH = res.tile([128, 128], F16)
            make_identity(nc, identH)
            ones_row = res.tile([1, 128], F16)
            nc.vector.memset(ones_row, 1.0)

            pwf = res.tile([128, 2, C], F32)
            nc.sync.dma_start(out=pwf, in_=pw[:].rearrange("(h p) c -> p h c", p=128))
            rwf = res.tile([128, 2, C], F32)
            nc.sync.dma_start(out=rwf, in_=rw[:].rearrange("(h p) c -> p h c", p=128))
            pbf = res.tile([128, 2], F32)
            nc.sync.dma_start(out=pbf, in_=pb[:].rearrange("(h p) -> p h", p=128))
            pb_row32 = res.tile([1, C], F32)
            nc.sync.dma_start(out=pb_row32, in_=pb[:].unsqueeze(0))
            rb_row32 = res.tile([1, C], F32)
            nc.sync.dma_start(out=rb_row32, in_=rb[:].unsqueeze(0))

            pw_n = res.tile([128, 2, C], F16)   # pw rows: [c' partition, c free]
            nc.vector.tensor_copy(pw_n.rearrange("p h c -> p (h c)"),
                                  pwf.rearrange("p h c -> p (h c)"))
            rw_n = res.tile([128, 2, C], F16)
            nc.vector.tensor_copy(rw_n.rearrange("p h c -> p (h c)"),
                                  rwf.rearrange("p h c -> p (h c)"))
            pb_col = res.tile([128, 2], F16)
            nc.vector.tensor_copy(pb_col, pbf)
            pb_row = res.tile([1, C], F16)
            nc.vector.tensor_copy(pb_row, pb_row32)
            rb_row = res.tile([1, C], F16)
            nc.vector.tensor_copy(rb_row, rb_row32)

            # transposed weights pwT[h] = proj_w.T rows h*128.. ([c partition, c' free])
            pwT = res.tile([128, 2, C], F16)
            rwT = res.tile([128, 2, C], F16)
            for src, dstT in ((pw_n, pwT), (rw_n, rwT)):
                for kh in range(2):
                    for mh in range(2):
                        tp = ps_tp.tile([128, 256], F16, tag="tpH", name="tp")
                        nc.tensor.transpose(tp[:, 0:128],
                                            src[:, kh, mh * 128:(mh + 1) * 128], identH)
                        dst = dstT[:, mh, kh * 128:(kh + 1) * 128]
                        if (kh + mh) % 2:
                            nc.vector.tensor_copy(dst, tp[:, 0:128])
                        else:
                            nc.scalar.activation(dst, tp[:, 0:128], AF.Copy)

            # u = proj_w.T @ proj_b (norm identity), as fp16 column halves
            ucol = res.tile([128, 2, 1], F16)
            for mh in range(2):
                u_ps = ps_acc.tile([1, 128], F32, tag="acc", name="u_ps")
                nc.tensor.matmul(u_ps, pb_col[:, 0:1], pw_n[:, 0, mh * 128:(mh + 1) * 128],
                                 start=True, stop=False)
                nc.tensor.matmul(u_ps, pb_col[:, 1:2], pw_n[:, 1, mh * 128:(mh + 1) * 128],
                                 start=False, stop=True)
                urow = itp.tile([1, 128], F16, tag="urow")
                nc.vector.tensor_copy(urow, u_ps)
                ut_ps = ps_tp.tile([128, NK], F16, tag="tpH", name="ut_ps")
                nc.tensor.transpose(ut_ps[:, 0:1], urow, identH[0:1, 0:1])
                nc.vector.tensor_copy(ucol[:, mh], ut_ps[:, 0:1])

            # beta = ||proj_b||^2 broadcast to all partitions
            pbsq = itp.tile([1, C], F32, tag="pbsq")
            beta1 = itp.tile([1, 1], F32, tag="beta1")
            nc.vector.scalar_tensor_tensor(pbsq, pb_row32, 1.0, pb_row32,
                                           op0=ALU.mult, op1=ALU.mult, accum_out=beta1)
            beta16 = itp.tile([1, 1], F16, tag="beta16")
            nc.vector.tensor_copy(beta16, beta1)
            bb_ps = ps_acc.tile([128, 1], F32, tag="acc", name="bb_ps")
            nc.tensor.matmul(bb_ps, ones_row, beta16)
            betab = res.tile([128, 1], F32)
            nc.vector.tensor_copy(betab, bb_ps)

            # residents
            fT = res.tile([128, 2, p], F16)             # F_p.T halves (c on partitions)
            natb = res.tile([128, nchunk, C + 1], F16)  # point-major copy + ones col
            nc.vector.memset(natb[:, :, C], 1.0)
            crossall = res.tile([128, nchunk], F32)     # F_p . u per point
            n2all = res.tile([128, nchunk], F32)        # ||F_p @ pw.T||^2 per point
            inv10 = res.tile([128, nchunk], F32)        # 10 / ||nodes_p||
            # softmax weights; 64-wide slots so 2 chunks transpose per xbar DMA
            # (transposed base partitions 0/64 — 96 is the broken quadrant);
            # col 8 = 1 folds refine_b into the cluster matmul via Dm9 row 8
            wgtall = res.tile([128, nchunk, 64], F16)
            nc.vector.memset(wgtall.rearrange("p n k -> p (n k)"), 0.0)
            nc.vector.memset(wgtall[:, :, NK], 1.0)

            # ---------------- initial centers ----------------
            g32 = scr.tile([NK, C], F32, tag="g32")
            for k, g in enumerate(idx):
                nc.sync.dma_start(out=g32[k:k + 1, :], in_=fp[:][g:g + 1, :])
            g16 = scr.tile([NK, C], F16, tag="g16")
            nc.vector.tensor_copy(g16, g32)
            g16T = itp.tile([128, 2, NK], F16, tag="g16T")
            for h in range(2):
                t_ps = ps_tp.tile([128, NK], F16, tag="tpH", name="t_ps")
                nc.tensor.transpose(t_ps, g16[:, h * 128:(h + 1) * 128],
                                    identH[0:NK, 0:NK])
                nc.vector.tensor_copy(g16T[:, h], t_ps)
            c0_ps = ps_acc.tile([NK, C], F32, tag="acc", name="c0_ps")
            nc.tensor.matmul(c0_ps, g16T[:, 0], pwT[:, 0], start=True, stop=False)
            nc.tensor.matmul(c0_ps, g16T[:, 1], pwT[:, 1], start=False, stop=False)
            nc.tensor.matmul(c0_ps, ones_row[:, 0:NK], pb_row, start=False, stop=True)
            centers = itp.tile([NK, C], F32, tag="centers")
            nc.vector.tensor_copy(centers, c0_ps)

            def make_G(centers_sb):
                """centers (8,C) f32 -> Gx [128, 2, 9] fp16 (cols 0:8 =
                proj_w.T @ l2(centers), col 8 = u) + hrow72 [1, GRP, 9] fp16
                (h_k = pb . cn_k tiled GRP times, 0 in each 9th col)."""
                csq = itp.tile([NK, C], F32, tag="csq")
                cn2 = itp.tile([NK, 1], F32, tag="cn2")
                nc.vector.scalar_tensor_tensor(csq, centers_sb, 1.0, centers_sb,
                                               op0=ALU.mult, op1=ALU.mult,
                                               accum_out=cn2)
                cnr = itp.tile([NK, 1], F32, tag="cnr")
                nc.scalar.activation(cnr, cn2, AF.Sqrt)
                rin = itp.tile([NK, 1], F32, tag="rin")
                nc.vector.reciprocal(rin, cnr)
                cn16 = itp.tile([NK, C], F16, tag="cn16")
                nc.vector.tensor_scalar_mul(cn16, centers_sb, rin)
                cnT = itp.tile([128, 2, NK], F16, tag="cnT")
                for h in range(2):
                    t_ps = ps_tp.tile([128, NK], F16, tag="tpH", name="t_ps")
                    nc.tensor.transpose(t_ps, cn16[:, h * 128:(h + 1) * 128],
                                        identH[0:NK, 0:NK])
                    nc.vector.tensor_copy(cnT[:, h], t_ps)
                Gx = itp.tile([128, 2, NK + 1], F16, tag="Gx")
                for mh in range(2):
                    g_ps = ps_acc.tile([128, NK], F32, tag="acc", name="g_ps")
                    nc.tensor.matmul(g_ps, pw_n[:, 0, mh * 128:(mh + 1) * 128],
                                     cnT[:, 0], start=True, stop=False)
                    nc.tensor.matmul(g_ps, pw_n[:, 1, mh * 128:(mh + 1) * 128],
                                     cnT[:, 1], start=False, stop=True)
                    nc.vector.tensor_copy(Gx[:, mh, 0:NK], g_ps)
                    nc.vector.tensor_copy(Gx[:, mh, NK:NK + 1], ucol[:, mh])
                h_ps = ps_acc.tile([1, NK], F32, tag="acc", name="h_ps")
                nc.tensor.matmul(h_ps, pb_col[:, 0:1], cnT[:, 0], start=True, stop=False)
                nc.tensor.matmul(h_ps, pb_col[:, 1:2], cnT[:, 1], start=False, stop=True)
                hrow9 = itp.tile([1, NK + 1], F16, tag="hrow9")
                nc.vector.tensor_copy(hrow9[:, 0:NK], h_ps)
                nc.vector.memset(hrow9[:, NK:NK + 1], 0.0)
                hrow72 = itp.tile([1, GRP, NK + 1], F16, tag="hrow72")
                nc.vector.tensor_copy(
                    hrow72, hrow9[:, None, :].broadcast_to([1, GRP, NK + 1]))
                return Gx, hrow72

            def update_centers(S_ps):
                """S_ps [8, 257] psum -> centers = (S/clamp(cnt,1)) @ pw.T + pb."""
                cnt = itp.tile([NK, 1], F32, tag="cnt")
                nc.vector.tensor_scalar(cnt, S_ps[:, C:C + 1], 1.0, None, op0=ALU.max)
                nc.vector.reciprocal(cnt, cnt)
                fm16 = itp.tile([NK, C], F16, tag="fm16")
                nc.vector.tensor_scalar_mul(fm16, S_ps[:, 0:C], cnt)
                fmT = itp.tile([128, 2, NK], F16, tag="fmT")
                for h in range(2):
                    t_ps = ps_tp.tile([128, NK], F16, tag="tpH", name="t_ps")
                    nc.tensor.transpose(t_ps, fm16[:, h * 128:(h + 1) * 128],
                                        identH[0:NK, 0:NK])
                    nc.vector.tensor_copy(fmT[:, h], t_ps)
                cp = ps_acc.tile([NK, C], F32, tag="acc", name="cp")
                nc.tensor.matmul(cp, fmT[:, 0], pwT[:, 0], start=True, stop=False)
                nc.tensor.matmul(cp, fmT[:, 1], pwT[:, 1], start=False, stop=False)
                nc.tensor.matmul(cp, ones_row[:, 0:NK], pb_row, start=False, stop=True)
                centers = itp.tile([NK, C], F32, tag="centers")
                nc.vector.tensor_copy(centers, cp)
                return centers

            # ---- group helpers ----
            def sim_group(gi, Gx, hrow72, norm_parity=None):
                """Sims (+h, +u cross col) for chunks gi*GRP..+GRP into one
                psum tile [128, GRP, 9]. For chunks with ci%2 == norm_parity,
                also emit nodes matmuls (sharing the sim LDWEIGHTS) and the
                square-accumulate that feeds the softmax temperature."""
                sg = ps_sg.tile([128, GRP, NK + 1], F32, tag="sg", name="sg")
                nc.tensor.matmul(sg.rearrange("p g k -> p (g k)"), ones_row,
                                 hrow72.rearrange("o g k -> o (g k)"),
                                 start=True, stop=False, skip_group_check=True)
                for i in range(GRP):
                    ci = gi * GRP + i
                    sl = slice(ci * 128, (ci + 1) * 128)
                    opN = None
                    if norm_parity is not None and ci % 2 == norm_parity:
                        opN = ps_acc.tile([128, 256], F32, tag="acc", name="opN")
                    for h in range(2):
                        nc.tensor.matmul(sg[:, i, :], fT[:, h, sl], Gx[:, h],
                                         start=False, stop=(h == 1),
                                         skip_group_check=True)
                        if opN is not None:
                            nc.tensor.matmul(opN, fT[:, h, sl], pwT[:, h],
                                             start=(h == 0), stop=(h == 1))
                    if opN is not None:
                        sq = scr.tile([128, 256], F16, tag="sq")
                        nc.scalar.activation(sq, opN, AF.Square,
                                             accum_out=n2all[:, ci:ci + 1])
                return sg

            def argmax_group(gi, sg, cross_to=None):
                """PSUM->SBUF copy, pairwise-max tree + is_ge onehot for a
                sim group. Returns ohg [128, GRP, NK] fp16."""
                sgs = sml.tile([128, GRP, NK + 1], F32, tag="sgs")
                nc.vector.tensor_copy(sgs.rearrange("p g k -> p (g k)"),
                                      sg.rearrange("p g k -> p (g k)"))
                m4 = sml.tile([128, GRP, 4], F32, tag="m4")
                nc.vector.tensor_tensor(m4, sgs[:, :, 0:4], sgs[:, :, 4:8], op=ALU.max)
                m2 = sml.tile([128, GRP, 2], F32, tag="m2")
                nc.vector.tensor_tensor(m2, m4[:, :, 0:2], m4[:, :, 2:4], op=ALU.max)
                m1 = sml.tile([128, GRP, 1], F32, tag="m1")
                nc.vector.tensor_tensor(m1, m2[:, :, 0:1], m2[:, :, 1:2], op=ALU.max)
                ohg = sml.tile([128, GRP, NK], F16, tag="ohg")
                nc.vector.tensor_tensor(ohg, sgs[:, :, 0:NK],
                                        m1.broadcast_to([128, GRP, NK]),
                                        op=ALU.is_ge)
                if cross_to is not None:
                    nc.scalar.activation(cross_to, sgs[:, :, NK], AF.Copy)
                return ohg

            def S_group(gi, ohg, S_ps):
                for i in range(GRP):
                    ci = gi * GRP + i
                    nc.tensor.matmul(S_ps, ohg[:, i, :], natb[:, ci, :],
                                     start=(ci == 0), stop=(ci == nchunk - 1))

            # =============== phase 1: load + transpose + iteration 1 ===============
            Gx, hrow72 = make_G(centers)
            S_ps = ps_s.tile([NK, C + 1], F32, tag="S")
            pend = []   # (gi, ohg) pending S matmul groups
            for gi in range(ngrp):
                nt = natp.tile([128, GRP, C], F32, tag="nat")
                nc.sync.dma_start(out=nt, in_=fp_v[:, gi * GRP:(gi + 1) * GRP, :])
                # batched f32->fp16 cast of the natural-layout group
                nc.gpsimd.tensor_copy(natb[:, gi * GRP:(gi + 1) * GRP, 0:128],
                                      nt[:, :, 0:128])
                nc.vector.tensor_copy(natb[:, gi * GRP:(gi + 1) * GRP, 128:256],
                                      nt[:, :, 128:256])
                for i in range(GRP):
                    ci = gi * GRP + i
                    sl = slice(ci * 128, (ci + 1) * 128)
                    nc.scalar.dma_start_transpose(fT[:, :, sl], natb[:, ci, 0:256])
                sg = sim_group(gi, Gx, hrow72)
                ohg = argmax_group(gi, sg)
                pend.append((gi, ohg))
                if len(pend) > 1:
                    S_group(*pend.pop(0), S_ps)
            S_group(*pend.pop(0), S_ps)
            centers = update_centers(S_ps)

            # =============== iterations 2..NUM_ITERS (+ split norm pass) ===============
            for it in range(1, NUM_ITERS):
                it_last = it == NUM_ITERS - 1
                Gx, hrow72 = make_G(centers)
                S_ps = ps_s.tile([NK, C + 1], F32, tag="S")
                pend = []
                # norm chunks: odd chunks in iter 2, even in iter 3
                parity = 0 if it_last else 1
                for gi in range(ngrp):
                    sg = sim_group(gi, Gx, hrow72, norm_parity=parity)
                    ohg = argmax_group(
                        gi, sg,
                        cross_to=crossall[:, gi * GRP:(gi + 1) * GRP] if it_last else None)
                    pend.append((gi, ohg))
                    if len(pend) > 1:
                        S_group(*pend.pop(0), S_ps)
                S_group(*pend.pop(0), S_ps)
                centers = update_centers(S_ps)

            # batched softmax scale: inv10 = 10 / max(sqrt(n2 + 2*cross + beta), eps)
            nrm = scr.tile([128, nchunk], F32, tag="nrm")
            nc.vector.scalar_tensor_tensor(nrm, crossall, 2.0, n2all,
                                           op0=ALU.mult, op1=ALU.add)
            nc.scalar.activation(nrm, nrm, AF.Sqrt, bias=betab)
            nc.vector.tensor_scalar(nrm, nrm, EPS, 0.1, op0=ALU.max, op1=ALU.mult)
            nc.vector.reciprocal(inv10, nrm)

            # =============== phase 5: final weights + refine ===============
            Gx, hrow72 = make_G(centers)
            # Dm9 = [centers @ refine_w.T ; refine_b] via selector row
            e9 = res.tile([1, NK + 1], F16)
            nc.vector.memset(e9[:, 0:NK], 0.0)
            nc.vector.memset(e9[:, NK:NK + 1], 1.0)
            cf16 = itp.tile([NK, C], F16, tag="cf16")
            nc.vector.tensor_copy(cf16, centers)
            cfT9 = itp.tile([128, 2, NK + 1], F16, tag="cfT9")
            nc.vector.memset(cfT9.rearrange("p h k -> p (h k)"), 0.0)
            for h in range(2):
                t_ps = ps_tp.tile([128, NK], F16, tag="tpH", name="t_ps")
                nc.tensor.transpose(t_ps, cf16[:, h * 128:(h + 1) * 128],
                                    identH[0:NK, 0:NK])
                nc.vector.tensor_copy(cfT9[:, h, 0:NK], t_ps)
            dm_ps = ps_acc.tile([NK + 1, C], F32, tag="acc", name="dm_ps")
            nc.tensor.matmul(dm_ps, cfT9[:, 0], rwT[:, 0], start=True, stop=False)
            nc.tensor.matmul(dm_ps, cfT9[:, 1], rwT[:, 1], start=False, stop=False)
            nc.tensor.matmul(dm_ps, e9, rb_row, start=False, stop=True)
            # replicate at partition offsets 0/64 so the cluster matmul's lhsT
            # (a slice of the xbar-transposed weights) pairs with an rhs at the
            # same base partition
            Dm9 = res.tile([128, C], F16)
            for q in range(2):
                nc.vector.tensor_copy(Dm9[q * 64:q * 64 + NK + 1, :], dm_ps)

            # sweep A: sims -> batched softmax -> weights (wgtall)
            for gi in range(ngrp):
                sg = sim_group(gi, Gx, hrow72)
                gsl = slice(gi * GRP, (gi + 1) * GRP)
                scl = sml.tile([128, GRP, NK], F32, tag="scl")
                nc.vector.tensor_tensor(
                    scl, sg[:, :, 0:NK],
                    inv10[:, gsl][:, :, None].broadcast_to([128, GRP, NK]),
                    op=ALU.mult)
                eg = sml.tile([128, GRP, NK], F32, tag="eg")
                nc.scalar.activation(eg.rearrange("p g k -> p (g k)"),
                                     scl.rearrange("p g k -> p (g k)"), AF.Exp)
                a4 = sml.tile([128, GRP, 4], F32, tag="a4")
                nc.vector.tensor_tensor(a4, eg[:, :, 0:4], eg[:, :, 4:8], op=ALU.add)
                a2 = sml.tile([128, GRP, 2], F32, tag="a2")
                nc.vector.tensor_tensor(a2, a4[:, :, 0:2], a4[:, :, 2:4], op=ALU.add)
                dn = sml.tile([128, GRP, 1], F32, tag="dn")
                nc.vector.tensor_tensor(dn, a2[:, :, 0:1], a2[:, :, 1:2], op=ALU.add)
                rdn = sml.tile([128, GRP, 1], F32, tag="rdn")
                nc.vector.reciprocal(rdn.rearrange("p g o -> p (g o)"),
                                     dn.rearrange("p g o -> p (g o)"))
                nc.vector.tensor_tensor(
                    wgtall[:, gsl, 0:NK], eg,
                    rdn.broadcast_to([128, GRP, NK]), op=ALU.mult)

            # sweep B: xbar weight transposes + refine + cluster matmul + output
            wT_sb = {}

            def transp_pair(q):
                # transpose 2 chunks' [128, 64] weight slots in one xbar DMA
                w9T = sml.tile([128, 128], F16, tag="w9T")
                nc.scalar.dma_start_transpose(w9T, wgtall[:, 2 * q:2 * (q + 1), :]
                                              .rearrange("p n k -> p (n k)"))
                wT_sb[q] = w9T

            for q in range(4):
                transp_pair(q)
            for gi in range(ngrp):
                for q in range(4 * gi + 4, 4 * gi + 8):
                    if q < 4 * ngrp:
                        transp_pair(q)
                for half in range(2):
                    ot = outp.tile([128, 4, C], F32, tag="ot", name="ot")
                    for i in range(4):
                        ci = gi * GRP + half * 4 + i
                        sl = slice(ci * 128, (ci + 1) * 128)
                        w9T = wT_sb[ci // 2]
                        opR = ps_acc.tile([128, C], F32, tag="acc", name="opR")
                        nc.tensor.matmul(opR, fT[:, 0, sl], rwT[:, 0],
                                         start=True, stop=False)
                        nc.tensor.matmul(opR, fT[:, 1, sl], rwT[:, 1],
                                         start=False, stop=False)
                        q = ci % 2
                        nc.tensor.matmul(opR, w9T[q * 64:q * 64 + NK + 1, :],
                                         Dm9[q * 64:q * 64 + NK + 1, :],
                                         start=False, stop=True, skip_group_check=True)
                        if i % 2 == 0:
                            nc.vector.tensor_copy(ot[:, i, :], opR)
                        else:
                            nc.scalar.activation(ot[:, i, :], opR, AF.Copy)
                        if ci % 2 == 1:
                            wT_sb.pop(ci // 2)
                    nc.sync.dma_start(
                        out=out_v[:, gi * GRP + half * 4:gi * GRP + half * 4 + 4, :],
                        in_=ot)

    nc.compile()
    return nc


_NC = None
TRACE = False
TRACE_DIR = None
LAST_EXEC_NS = None


def kernel(F_p, proj_w, proj_b, refine_w, refine_b):
    global _NC, LAST_EXEC_NS
    if _NC is None:
        _NC = build_bass()
    F_p = np.ascontiguousarray(F_p, dtype=np.float32)
    shared = {
        "pw": np.ascontiguousarray(proj_w, dtype=np.float32),
        "pb": np.ascontiguousarray(proj_b, dtype=np.float32),
        "rw": np.ascontiguousarray(refine_w, dtype=np.float32),
        "rb": np.ascontiguousarray(refine_b, dtype=np.float32),
    }
    in_maps = [{"fp": F_p[i], **shared} for i in range(N_CORES)]
    res = run_bass_kernel_spmd(_NC, in_maps, list(range(N_CORES)), trace=TRACE,
                               tmpdir=TRACE_DIR)
    LAST_EXEC_NS = res.exec_time_ns
    return np.stack([res.results[i]["out"] for i in range(N_CORES)], axis=0)


# revision 25
# speedup vs baseline: 1.7555x; 1.7555x over previous
"""NodeClustering (vq_codebook) Trainium2 kernel — v3.

Math (per batch element b, P=16384 points, C=256 channels, K=8 clusters):
  nodes = F_p @ proj_w.T + proj_b
  3 iterations of: sim = l2(nodes) @ l2(centers).T ; assign = argmax;
                   centers = segment_mean(nodes)
  weights = softmax(10 * l2(nodes) @ l2(centers).T)
  out = (weights@centers + F_p) @ refine_w.T + refine_b

Restructuring so `nodes` is never materialized:
  nodes[p] . cn_k       = F_p[p] . g_k + h_k   (g_k = proj_w.T @ cn_k, h_k = proj_b . cn_k)
  segment_mean(nodes)   = segment_mean(F_p) @ proj_w.T + proj_b
  out                   = F_p @ refine_w.T + weights @ [centers @ refine_w.T ; refine_b]
  iteration argmax is scale-invariant -> skip 1/||nodes_p||; final softmax
  needs inv10_p = 10/||nodes_p||, with
  ||nodes_p||^2 = ||F_p[p] @ proj_w.T||^2 + 2*F_p[p].u + ||proj_b||^2,  u = proj_w.T @ proj_b

v3 performance structure (v2 at 614us was bound by per-chunk DVE/ACT/GPSIMD
small ops, which cost ~400-900ns each on HW regardless of size):
  - fp16 residents: fT (C-major, [128, 2, P]) + natb (point-major + baked
    ones column for counts). Iterations re-read nothing from HBM.
  - Chunks processed in groups of GRP=8. Sims for a group land in ONE psum
    tile [128, 8, 9]; h is added by a single ones x hrow matmul per group
    (PE) instead of per-chunk vector adds; argmax+onehot is 4 batched DVE
    ops per group (pairwise-max tree + one is_ge against a stride-0
    broadcast of the max), not 3 ops per chunk.
  - Norm pass (nodes + square-accumulate) split across iterations 2 and 3
    (it only depends on F_p), sharing the sim LDWEIGHTS.
  - Final softmax fully batched per group: scale-by-inv10 (broadcast mult),
    exp, avg-pool denominator, reciprocal, normalize — 5 vector ops per 8
    chunks. Phase 5 runs as sweep A (sims+softmax) then sweep B (weight
    transposes + refine + cluster matmul + output) within 8 PSUM banks.
  - f32->fp16 casts batched per 8-chunk DMA group, split GPSIMD/DVE.

Sharding: pure data parallel, core i <- batch element i (B=8, 8 cores).
"""

import sys
import numpy as np

sys.path.insert(0, "/opt/trn_rl_repo")

import concourse.bass as bass
import concourse.bacc as bacc
import concourse.mybir as mybir
import concourse.tile as tile
from concourse._compat import get_trn_type
from concourse.bass_utils import axon_active
from concourse.masks import make_identity
from concourse.bass_utils import run_bass_kernel_spmd

P = 16384
C = 256
NK = 8
NUM_ITERS = 3
EPS = 1e-12
N_CORES = 8
NCHUNK = P // 128
GRP = 8          # chunks per sim/argmax group (= chunks per input/output DMA)

F32 = mybir.dt.float32
F16 = mybir.dt.float16
AF = mybir.ActivationFunctionType
ALU = mybir.AluOpType


def build_bass(p=P):
    nchunk = p // 128
    ngrp = nchunk // GRP
    idx = list(np.linspace(0, p - 1, NK).astype(np.int64))
    nc = bacc.Bacc(
        get_trn_type() or "TRN2",
        target_bir_lowering=False,
        debug=not axon_active(),
        num_devices=N_CORES,
    )

    fp = nc.dram_tensor("fp", [p, C], F32, kind="ExternalInput")
    pw = nc.dram_tensor("pw", [C, C], F32, kind="ExternalInput")
    pb = nc.dram_tensor("pb", [C], F32, kind="ExternalInput")
    rw = nc.dram_tensor("rw", [C, C], F32, kind="ExternalInput")
    rb = nc.dram_tensor("rb", [C], F32, kind="ExternalInput")
    out = nc.dram_tensor("out", [p, C], F32, kind="ExternalOutput")

    fp_v = fp[:].rearrange("(n p) c -> p n c", p=128)
    out_v = out[:].rearrange("(n p) c -> p n c", p=128)

    with tile.TileContext(nc) as tc:
        with (
            tc.tile_pool(name="res", bufs=1) as res,
            tc.tile_pool(name="natp", bufs=2) as natp,
            tc.tile_pool(name="outp", bufs=2) as outp,
            tc.tile_pool(name="sml", bufs=4) as sml,
            tc.tile_pool(name="scr", bufs=2) as scr,
            tc.tile_pool(name="itp", bufs=2) as itp,
            tc.tile_pool(name="ps_tp", bufs=2, space="PSUM") as ps_tp,    # fp16: transposes / wTg
            tc.tile_pool(name="ps_acc", bufs=3, space="PSUM") as ps_acc,  # f32: opN/opR + small mm
            tc.tile_pool(name="ps_sg", bufs=2, space="PSUM") as ps_sg,    # sim groups
            tc.tile_pool(name="ps_s", bufs=1, space="PSUM") as ps_s,      # S accumulator
        ):
            # ---------------- phase 0: constants + weights ----------------
            identH = res.tile([128, 128], F16)
            make_identity(nc, identH)
            ones_row = res.tile([1, 128], F16)
            nc.vector.memset(ones_row, 1.0)

            pwf = res.tile([128, 2, C], F32)
            nc.sync.dma_start(out=pwf, in_=pw[:].rearrange("(h p) c -> p h c", p=128))
            rwf = res.tile([128, 2, C], F32)
            nc.sync.dma_start(out=rwf, in_=rw[:].rearrange("(h p) c -> p h c", p=128))
            pbf = res.tile([128, 2], F32)
            nc.sync.dma_start(out=pbf, in_=pb[:].rearrange("(h p) -> p h", p=128))
            pb_row32 = res.tile([1, C], F32)
            nc.sync.dma_start(out=pb_row32, in_=pb[:].unsqueeze(0))
            rb_row32 = res.tile([1, C], F32)
            nc.sync.dma_start(out=rb_row32, in_=rb[:].unsqueeze(0))

            pw_n = res.tile([128, 2, C], F16)   # pw rows: [c' partition, c free]
            nc.vector.tensor_copy(pw_n.rearrange("p h c -> p (h c)"),
                                  pwf.rearrange("p h c -> p (h c)"))
            rw_n = res.tile([128, 2, C], F16)
            nc.vector.tensor_copy(rw_n.rearrange("p h c -> p (h c)"),
                                  rwf.rearrange("p h c -> p (h c)"))
            pb_col = res.tile([128, 2], F16)
            nc.vector.tensor_copy(pb_col, pbf)
            pb_row = res.tile([1, C], F16)
            nc.vector.tensor_copy(pb_row, pb_row32)
            rb_row = res.tile([1, C], F16)
            nc.vector.tensor_copy(rb_row, rb_row32)

            # transposed weights pwT[h] = proj_w.T rows h*128.. ([c partition, c' free])
            pwT = res.tile([128, 2, C], F16)
            rwT = res.tile([128, 2, C], F16)
            for src, dstT in ((pw_n, pwT), (rw_n, rwT)):
                for kh in range(2):
                    for mh in range(2):
                        tp = ps_tp.tile([128, 256], F16, tag="tpH", name="tp")
                        nc.tensor.transpose(tp[:, 0:128],
                                            src[:, kh, mh * 128:(mh + 1) * 128], identH)
                        dst = dstT[:, mh, kh * 128:(kh + 1) * 128]
                        if (kh + mh) % 2:
                            nc.vector.tensor_copy(dst, tp[:, 0:128])
                        else:
                            nc.scalar.activation(dst, tp[:, 0:128], AF.Copy)

            # u = proj_w.T @ proj_b (norm identity), as fp16 column halves
            ucol = res.tile([128, 2, 1], F16)
            for mh in range(2):
                u_ps = ps_acc.tile([1, 128], F32, tag="acc", name="u_ps")
                nc.tensor.matmul(u_ps, pb_col[:, 0:1], pw_n[:, 0, mh * 128:(mh + 1) * 128],
                                 start=True, stop=False)
                nc.tensor.matmul(u_ps, pb_col[:, 1:2], pw_n[:, 1, mh * 128:(mh + 1) * 128],
                                 start=False, stop=True)
                urow = itp.tile([1, 128], F16, tag="urow")
                nc.vector.tensor_copy(urow, u_ps)
                ut_ps = ps_tp.tile([128, NK], F16, tag="tpH", name="ut_ps")
                nc.tensor.transpose(ut_ps[:, 0:1], urow, identH[0:1, 0:1])
                nc.vector.tensor_copy(ucol[:, mh], ut_ps[:, 0:1])

            # beta = ||proj_b||^2 broadcast to all partitions
            pbsq = itp.tile([1, C], F32, tag="pbsq")
            beta1 = itp.tile([1, 1], F32, tag="beta1")
            nc.vector.scalar_tensor_tensor(pbsq, pb_row32, 1.0, pb_row32,
                                           op0=ALU.mult, op1=ALU.mult, accum_out=beta1)
            beta16 = itp.tile([1, 1], F16, tag="beta16")
            nc.vector.tensor_copy(beta16, beta1)
            bb_ps = ps_acc.tile([128, 1], F32, tag="acc", name="bb_ps")
            nc.tensor.matmul(bb_ps, ones_row, beta16)
            betab = res.tile([128, 1], F32)
            nc.vector.tensor_copy(betab, bb_ps)

            # residents
            fT = res.tile([128, 2, p], F16)             # F_p.T halves (c on partitions)
            natb = res.tile([128, nchunk, C + 1], F16)  # point-major copy + ones col
            nc.vector.memset(natb[:, :, C], 1.0)
            crossall = res.tile([128, nchunk], F32)     # F_p . u per point
            n2all = res.tile([128, nchunk], F32)        # ||F_p @ pw.T||^2 per point
            inv10 = res.tile([128, nchunk], F32)        # 10 / ||nodes_p||
            wgtall = res.tile([128, nchunk, NK + 1], F16)  # softmax weights + ones col
            nc.vector.memset(wgtall[:, :, NK], 1.0)

            # ---------------- initial centers ----------------
            g32 = scr.tile([NK, C], F32, tag="g32")
            for k, g in enumerate(idx):
                nc.sync.dma_start(out=g32[k:k + 1, :], in_=fp[:][g:g + 1, :])
            g16 = scr.tile([NK, C], F16, tag="g16")
            nc.vector.tensor_copy(g16, g32)
            g16T = itp.tile([128, 2, NK], F16, tag="g16T")
            for h in range(2):
                t_ps = ps_tp.tile([128, NK], F16, tag="tpH", name="t_ps")
                nc.tensor.transpose(t_ps, g16[:, h * 128:(h + 1) * 128],
                                    identH[0:NK, 0:NK])
                nc.vector.tensor_copy(g16T[:, h], t_ps)
            c0_ps = ps_acc.tile([NK, C], F32, tag="acc", name="c0_ps")
            nc.tensor.matmul(c0_ps, g16T[:, 0], pwT[:, 0], start=True, stop=False)
            nc.tensor.matmul(c0_ps, g16T[:, 1], pwT[:, 1], start=False, stop=False)
            nc.tensor.matmul(c0_ps, ones_row[:, 0:NK], pb_row, start=False, stop=True)
            centers = itp.tile([NK, C], F32, tag="centers")
            nc.vector.tensor_copy(centers, c0_ps)

            def make_G(centers_sb):
                """centers (8,C) f32 -> Gx [128, 2, 9] fp16 (cols 0:8 =
                proj_w.T @ l2(centers), col 8 = u) + hrow72 [1, GRP, 9] fp16
                (h_k = pb . cn_k tiled GRP times, 0 in each 9th col)."""
                csq = itp.tile([NK, C], F32, tag="csq")
                cn2 = itp.tile([NK, 1], F32, tag="cn2")
                nc.vector.scalar_tensor_tensor(csq, centers_sb, 1.0, centers_sb,
                                               op0=ALU.mult, op1=ALU.mult,
                                               accum_out=cn2)
                cnr = itp.tile([NK, 1], F32, tag="cnr")
                nc.scalar.activation(cnr, cn2, AF.Sqrt)
                rin = itp.tile([NK, 1], F32, tag="rin")
                nc.vector.reciprocal(rin, cnr)
                cn16 = itp.tile([NK, C], F16, tag="cn16")
                nc.vector.tensor_scalar_mul(cn16, centers_sb, rin)
                cnT = itp.tile([128, 2, NK], F16, tag="cnT")
                for h in range(2):
                    t_ps = ps_tp.tile([128, NK], F16, tag="tpH", name="t_ps")
                    nc.tensor.transpose(t_ps, cn16[:, h * 128:(h + 1) * 128],
                                        identH[0:NK, 0:NK])
                    nc.vector.tensor_copy(cnT[:, h], t_ps)
                Gx = itp.tile([128, 2, NK + 1], F16, tag="Gx")
                for mh in range(2):
                    g_ps = ps_acc.tile([128, NK], F32, tag="acc", name="g_ps")
                    nc.tensor.matmul(g_ps, pw_n[:, 0, mh * 128:(mh + 1) * 128],
                                     cnT[:, 0], start=True, stop=False)
                    nc.tensor.matmul(g_ps, pw_n[:, 1, mh * 128:(mh + 1) * 128],
                                     cnT[:, 1], start=False, stop=True)
                    nc.vector.tensor_copy(Gx[:, mh, 0:NK], g_ps)
                    nc.vector.tensor_copy(Gx[:, mh, NK:NK + 1], ucol[:, mh])
                h_ps = ps_acc.tile([1, NK], F32, tag="acc", name="h_ps")
                nc.tensor.matmul(h_ps, pb_col[:, 0:1], cnT[:, 0], start=True, stop=False)
                nc.tensor.matmul(h_ps, pb_col[:, 1:2], cnT[:, 1], start=False, stop=True)
                hrow9 = itp.tile([1, NK + 1], F16, tag="hrow9")
                nc.vector.tensor_copy(hrow9[:, 0:NK], h_ps)
                nc.vector.memset(hrow9[:, NK:NK + 1], 0.0)
                hrow72 = itp.tile([1, GRP, NK + 1], F16, tag="hrow72")
                nc.vector.tensor_copy(
                    hrow72, hrow9[:, None, :].broadcast_to([1, GRP, NK + 1]))
                keep_warm()
                return Gx, hrow72

            def keep_warm(n=4):
                # HAM keep-warm: the PE array clock-gates to 1.2 GHz unless its
                # streaming duty stays high; these dummy matmuls (one identity
                # LDWEIGHTS + n 512-col streams from resident fT) bridge the
                # vector-engine-only stretches at phase boundaries.
                dps = ps_acc.tile([128, 512], F32, tag="acc", name="dps")
                for j in range(n):
                    nc.tensor.matmul(dps, identH, fT[:, 0, j * 512:(j + 1) * 512],
                                     start=(j == 0), stop=(j == n - 1),
                                     skip_group_check=True)

            def update_centers(S_ps):
                """S_ps [8, 257] psum -> centers = (S/clamp(cnt,1)) @ pw.T + pb."""
                cnt = itp.tile([NK, 1], F32, tag="cnt")
                nc.vector.tensor_scalar(cnt, S_ps[:, C:C + 1], 1.0, None, op0=ALU.max)
                nc.vector.reciprocal(cnt, cnt)
                fm16 = itp.tile([NK, C], F16, tag="fm16")
                nc.vector.tensor_scalar_mul(fm16, S_ps[:, 0:C], cnt)
                fmT = itp.tile([128, 2, NK], F16, tag="fmT")
                for h in range(2):
                    t_ps = ps_tp.tile([128, NK], F16, tag="tpH", name="t_ps")
                    nc.tensor.transpose(t_ps, fm16[:, h * 128:(h + 1) * 128],
                                        identH[0:NK, 0:NK])
                    nc.vector.tensor_copy(fmT[:, h], t_ps)
                cp = ps_acc.tile([NK, C], F32, tag="acc", name="cp")
                nc.tensor.matmul(cp, fmT[:, 0], pwT[:, 0], start=True, stop=False)
                nc.tensor.matmul(cp, fmT[:, 1], pwT[:, 1], start=False, stop=False)
                nc.tensor.matmul(cp, ones_row[:, 0:NK], pb_row, start=False, stop=True)
                keep_warm()
                centers = itp.tile([NK, C], F32, tag="centers")
                nc.vector.tensor_copy(centers, cp)
                return centers

            # ---- group helpers ----
            def sim_group(gi, Gx, hrow72, norm_parity=None):
                """Sims (+h, +u cross col) for chunks gi*GRP..+GRP into one
                psum tile [128, GRP, 9]. For chunks with ci%2 == norm_parity,
                also emit nodes matmuls (sharing the sim LDWEIGHTS) and the
                square-accumulate that feeds the softmax temperature."""
                sg = ps_sg.tile([128, GRP, NK + 1], F32, tag="sg", name="sg")
                nc.tensor.matmul(sg.rearrange("p g k -> p (g k)"), ones_row,
                                 hrow72.rearrange("o g k -> o (g k)"),
                                 start=True, stop=False, skip_group_check=True)
                for i in range(GRP):
                    ci = gi * GRP + i
                    sl = slice(ci * 128, (ci + 1) * 128)
                    opN = None
                    if norm_parity is not None and ci % 2 == norm_parity:
                        opN = ps_acc.tile([128, 256], F32, tag="acc", name="opN")
                    for h in range(2):
                        nc.tensor.matmul(sg[:, i, :], fT[:, h, sl], Gx[:, h],
                                         start=False, stop=(h == 1),
                                         skip_group_check=True)
                        if opN is not None:
                            nc.tensor.matmul(opN, fT[:, h, sl], pwT[:, h],
                                             start=(h == 0), stop=(h == 1))
                    if opN is not None:
                        sq = scr.tile([128, 256], F16, tag="sq")
                        nc.scalar.activation(sq, opN, AF.Square,
                                             accum_out=n2all[:, ci:ci + 1])
                return sg

            def argmax_group(gi, sg, cross_to=None):
                """PSUM->SBUF copy, pairwise-max tree + is_ge onehot for a
                sim group. Returns ohg [128, GRP, NK] fp16."""
                sgs = sml.tile([128, GRP, NK + 1], F32, tag="sgs")
                nc.vector.tensor_copy(sgs.rearrange("p g k -> p (g k)"),
                                      sg.rearrange("p g k -> p (g k)"))
                m4 = sml.tile([128, GRP, 4], F32, tag="m4")
                nc.vector.tensor_tensor(m4, sgs[:, :, 0:4], sgs[:, :, 4:8], op=ALU.max)
                m2 = sml.tile([128, GRP, 2], F32, tag="m2")
                nc.vector.tensor_tensor(m2, m4[:, :, 0:2], m4[:, :, 2:4], op=ALU.max)
                m1 = sml.tile([128, GRP, 1], F32, tag="m1")
                nc.vector.tensor_tensor(m1, m2[:, :, 0:1], m2[:, :, 1:2], op=ALU.max)
                ohg = sml.tile([128, GRP, NK], F16, tag="ohg")
                nc.vector.tensor_tensor(ohg, sgs[:, :, 0:NK],
                                        m1.broadcast_to([128, GRP, NK]),
                                        op=ALU.is_ge)
                if cross_to is not None:
                    nc.scalar.activation(cross_to, sgs[:, :, NK], AF.Copy)
                return ohg

            def S_group(gi, ohg, S_ps, pad=False):
                for i in range(GRP):
                    ci = gi * GRP + i
                    nc.tensor.matmul(S_ps[:, 0:C + 1], ohg[:, i, :], natb[:, ci, :],
                                     start=(ci == 0), stop=(ci == nchunk - 1))
                    if pad and i % 2 == 0:
                        # duty-raising repeat into the S bank's unused columns
                        # (same stationary -> no LDWEIGHTS; start=False because
                        # a start=True matmul clears the WHOLE bank, including
                        # the live S accumulation)
                        nc.tensor.matmul(S_ps[:, C + 1:512], ohg[:, i, :],
                                         natb[:, ci, 0:255], start=False, stop=False,
                                         skip_group_check=True)

            # =============== phase 1: load + transpose + iteration 1 ===============
            Gx, hrow72 = make_G(centers)
            S_ps = ps_s.tile([NK, 512], F32, tag="S")
            pend = []   # (gi, ohg) pending S matmul groups
            for gi in range(ngrp):
                nt = natp.tile([128, GRP, C], F32, tag="nat")
                nc.sync.dma_start(out=nt, in_=fp_v[:, gi * GRP:(gi + 1) * GRP, :])
                # batched f32->fp16 cast of the natural-layout group
                nc.gpsimd.tensor_copy(natb[:, gi * GRP:(gi + 1) * GRP, 0:128],
                                      nt[:, :, 0:128])
                nc.vector.tensor_copy(natb[:, gi * GRP:(gi + 1) * GRP, 128:256],
                                      nt[:, :, 128:256])
                for i in range(GRP):
                    ci = gi * GRP + i
                    sl = slice(ci * 128, (ci + 1) * 128)
                    tp = ps_tp.tile([128, 256], F16, tag="tpH", name="tp")
                    nc.tensor.transpose(tp[:, 0:128], natb[:, ci, 0:128], identH)
                    nc.tensor.transpose(tp[:, 128:256], natb[:, ci, 128:256], identH)
                    nc.scalar.activation(fT[:, :, sl],
                                         tp.rearrange("p (h x) -> p h x", h=2), AF.Copy)
                sg = sim_group(gi, Gx, hrow72)
                ohg = argmax_group(gi, sg)
                pend.append((gi, ohg))
                if len(pend) > 1:
                    S_group(*pend.pop(0), S_ps)
            S_group(*pend.pop(0), S_ps)
            centers = update_centers(S_ps)

            # =============== iterations 2..NUM_ITERS (+ split norm pass) ===============
            for it in range(1, NUM_ITERS):
                it_last = it == NUM_ITERS - 1
                Gx, hrow72 = make_G(centers)
                S_ps = ps_s.tile([NK, 512], F32, tag="S")
                pend = []
                # norm chunks: odd chunks in iter 2, even in iter 3
                parity = 0 if it_last else 1
                for gi in range(ngrp):
                    sg = sim_group(gi, Gx, hrow72, norm_parity=parity)
                    ohg = argmax_group(
                        gi, sg,
                        cross_to=crossall[:, gi * GRP:(gi + 1) * GRP] if it_last else None)
                    pend.append((gi, ohg))
                    if len(pend) > 1:
                        S_group(*pend.pop(0), S_ps, pad=True)
                S_group(*pend.pop(0), S_ps, pad=True)
                centers = update_centers(S_ps)

            # batched softmax scale: inv10 = 10 / max(sqrt(n2 + 2*cross + beta), eps)
            nrm = scr.tile([128, nchunk], F32, tag="nrm")
            nc.vector.scalar_tensor_tensor(nrm, crossall, 2.0, n2all,
                                           op0=ALU.mult, op1=ALU.add)
            nc.scalar.activation(nrm, nrm, AF.Sqrt, bias=betab)
            nc.vector.tensor_scalar(nrm, nrm, EPS, 0.1, op0=ALU.max, op1=ALU.mult)
            nc.vector.reciprocal(inv10, nrm)

            # =============== phase 5: final weights + refine ===============
            Gx, hrow72 = make_G(centers)
            # Dm9 = [centers @ refine_w.T ; refine_b] via selector row
            e9 = res.tile([1, NK + 1], F16)
            nc.vector.memset(e9[:, 0:NK], 0.0)
            nc.vector.memset(e9[:, NK:NK + 1], 1.0)
            cf16 = itp.tile([NK, C], F16, tag="cf16")
            nc.vector.tensor_copy(cf16, centers)
            cfT9 = itp.tile([128, 2, NK + 1], F16, tag="cfT9")
            nc.vector.memset(cfT9.rearrange("p h k -> p (h k)"), 0.0)
            for h in range(2):
                t_ps = ps_tp.tile([128, NK], F16, tag="tpH", name="t_ps")
                nc.tensor.transpose(t_ps, cf16[:, h * 128:(h + 1) * 128],
                                    identH[0:NK, 0:NK])
                nc.vector.tensor_copy(cfT9[:, h, 0:NK], t_ps)
            dm_ps = ps_acc.tile([NK + 1, C], F32, tag="acc", name="dm_ps")
            nc.tensor.matmul(dm_ps, cfT9[:, 0], rwT[:, 0], start=True, stop=False)
            nc.tensor.matmul(dm_ps, cfT9[:, 1], rwT[:, 1], start=False, stop=False)
            nc.tensor.matmul(dm_ps, e9, rb_row, start=False, stop=True)
            Dm9 = res.tile([NK + 1, C], F16)
            nc.vector.tensor_copy(Dm9, dm_ps)

            # sweep A: sims -> batched softmax -> weights (wgtall)
            for gi in range(ngrp):
                sg = sim_group(gi, Gx, hrow72)
                gsl = slice(gi * GRP, (gi + 1) * GRP)
                scl = sml.tile([128, GRP, NK], F32, tag="scl")
                nc.vector.tensor_tensor(
                    scl, sg[:, :, 0:NK],
                    inv10[:, gsl][:, :, None].broadcast_to([128, GRP, NK]),
                    op=ALU.mult)
                eg = sml.tile([128, GRP, NK], F32, tag="eg")
                nc.scalar.activation(eg.rearrange("p g k -> p (g k)"),
                                     scl.rearrange("p g k -> p (g k)"), AF.Exp)
                a4 = sml.tile([128, GRP, 4], F32, tag="a4")
                nc.vector.tensor_tensor(a4, eg[:, :, 0:4], eg[:, :, 4:8], op=ALU.add)
                a2 = sml.tile([128, GRP, 2], F32, tag="a2")
                nc.vector.tensor_tensor(a2, a4[:, :, 0:2], a4[:, :, 2:4], op=ALU.add)
                dn = sml.tile([128, GRP, 1], F32, tag="dn")
                nc.vector.tensor_tensor(dn, a2[:, :, 0:1], a2[:, :, 1:2], op=ALU.add)
                rdn = sml.tile([128, GRP, 1], F32, tag="rdn")
                nc.vector.reciprocal(rdn.rearrange("p g o -> p (g o)"),
                                     dn.rearrange("p g o -> p (g o)"))
                nc.vector.tensor_tensor(
                    wgtall[:, gsl, 0:NK], eg,
                    rdn.broadcast_to([128, GRP, NK]), op=ALU.mult)

            # sweep B: weight transposes + refine + cluster matmul + output
            wTg_sb = {}

            def transp_group(gi):
                wTg = ps_tp.tile([NK + 1, GRP, 128], F16, tag="tpH", name="wTg")
                for i in range(GRP):
                    ci = gi * GRP + i
                    nc.tensor.transpose(wTg[:, i, :], wgtall[:, ci, :], identH)
                w9 = sml.tile([NK + 1, GRP, 128], F16, tag="w9")
                nc.vector.tensor_copy(w9.rearrange("k g x -> k (g x)"),
                                      wTg.rearrange("k g x -> k (g x)"))
                wTg_sb[gi] = w9

            transp_group(0)
            for gi in range(ngrp):
                if gi + 1 < ngrp:
                    transp_group(gi + 1)
                w9 = wTg_sb.pop(gi)
                ot = outp.tile([128, GRP, C], F32, tag="ot", name="ot")
                for i in range(GRP):
                    ci = gi * GRP + i
                    sl = slice(ci * 128, (ci + 1) * 128)
                    opR = ps_acc.tile([128, C], F32, tag="acc", name="opR")
                    nc.tensor.matmul(opR, fT[:, 0, sl], rwT[:, 0],
                                     start=True, stop=False)
                    nc.tensor.matmul(opR, fT[:, 1, sl], rwT[:, 1],
                                     start=False, stop=False)
                    nc.tensor.matmul(opR, w9[:, i, :], Dm9,
                                     start=False, stop=True, skip_group_check=True)
                    if i % 2 == 0:
                        nc.vector.tensor_copy(ot[:, i, :], opR)
                    else:
                        nc.scalar.activation(ot[:, i, :], opR, AF.Copy)
                nc.sync.dma_start(out=out_v[:, gi * GRP:(gi + 1) * GRP, :], in_=ot)

    nc.compile()
    return nc


_NC = None
TRACE = False
TRACE_DIR = None
LAST_EXEC_NS = None


def kernel(F_p, proj_w, proj_b, refine_w, refine_b):
    global _NC, LAST_EXEC_NS
    if _NC is None:
        _NC = build_bass()
    F_p = np.ascontiguousarray(F_p, dtype=np.float32)
    shared = {
        "pw": np.ascontiguousarray(proj_w, dtype=np.float32),
        "pb": np.ascontiguousarray(proj_b, dtype=np.float32),
        "rw": np.ascontiguousarray(refine_w, dtype=np.float32),
        "rb": np.ascontiguousarray(refine_b, dtype=np.float32),
    }
    in_maps = [{"fp": F_p[i], **shared} for i in range(N_CORES)]
    res = run_bass_kernel_spmd(_NC, in_maps, list(range(N_CORES)), trace=TRACE,
                               tmpdir=TRACE_DIR)
    LAST_EXEC_NS = res.exec_time_ns
    return np.stack([res.results[i]["out"] for i in range(N_CORES)], axis=0)
